# revision 29
# baseline (speedup 1.0000x reference)
"""Trainium2 Bass kernel for BakaAttentionQKV.

Computes, for full inputs (B=4, T=2048, D=2048, 16 heads):
    q = q_state @ Wq.T ; k = k_state @ Wk.T ; v = v_state @ Wv.T
    q,k -> concat(x, negator - x) -> LayerNorm(gamma, beta) -> split heads -> RoPE(first 128 dims)
    v -> split heads
Returns (q [4,16,2048,256], k [4,16,2048,256], v [4,16,2048,128]).

Key algebraic simplification: for z = concat(x, negator - x),
mean(z) = negator/2 exactly, and var(z) = mean_f((x - negator/2)^2), so the
LayerNorm only needs the first half: n = (x - c) * rsqrt(var + eps) with
c = negator/2; the second half of the normalized vector is exactly -n.
With gamma == 1 and beta == 0 (the canonical inputs) the full LN output is
[n, -n], so the matmul for the second half is free.

Sharding: 8-way data parallel over the 8192 (b, t) token rows; each core gets
1024 contiguous rows (= half of one batch element) and computes all heads.
Matmuls run in fp16 (10-bit mantissa; full 1 cycle/row PE rate and half the
HBM traffic of fp32) with fp32 PSUM accumulation; x/W are converted on the
host. BAKA_MM_DTYPE=float32r selects a TF32-like fp32r path instead (slightly
more accurate, ~45% slower end-to-end because weights can't double-buffer).
Measured vs the fp64-ish jax reference: rel2 ~2.5e-4.
"""
import sys

sys.path.insert(0, "/opt/trn_rl_repo")

from contextlib import ExitStack

import numpy as np

import concourse.bacc as bacc
import concourse.bass as bass
import concourse.tile as tile
from concourse import mybir
from concourse.bass_utils import run_bass_kernel_spmd

F32 = mybir.dt.float32
F32R = mybir.dt.float32r
AF = mybir.ActivationFunctionType
OP = mybir.AluOpType

B, T, D = 4, 2048, 2048
NCORES = 8
TOK = B * T // NCORES          # 1024 token rows per core
NT = TOK // 128                # 8 token tiles per core
ND = D // 128                  # 16 contraction tiles
NFB = D // 512                 # 4 psum banks per row block
EPS = 1e-5
ROT = 128                      # rotary dims per head

# diagnostics for test harness
last_exec_time_ns = None
last_results = None


def _round_fp32r(a: np.ndarray) -> np.ndarray:
    """Round fp32 -> fp32r (11-bit mantissa, low 12 bits zero), RNE."""
    u = np.ascontiguousarray(a, dtype=np.float32).view(np.uint32).copy()
    u += np.uint32(0x7FF) + ((u >> np.uint32(12)) & np.uint32(1))
    u &= np.uint32(0xFFFFF000)
    return u.view(np.float32)


def _build(c_half: float, mmdt=None):
    if mmdt is None:
        mmdt = mybir.dt.float16
    nc = bacc.Bacc("TRN2", target_bir_lowering=False, debug=False)
    # register const APs needed by scalar.activation float biases
    for val in (-c_half, EPS):
        if (F32, val) not in nc.const_aps.aps:
            t = nc.alloc_sbuf_tensor(f"const-f32-{val}", [128, 1], F32)
            nc.gpsimd.memset(t.ap(), val)
            nc.const_aps.aps[(F32, val)] = t.ap()
    nc.all_engine_barrier()
    dram = {}
    for t in "qkv":
        # x pre-packed host-side as [tt, p, dt*128+tl]: each token tile's
        # stationary block loads as one contiguous 2D DMA
        dram[f"x{t}"] = nc.dram_tensor(f"x{t}", [NT, 128, D], mmdt, kind="ExternalInput")
        dram[f"w{t}"] = nc.dram_tensor(f"w{t}", [D, D], mmdt, kind="ExternalInput")
    cos_d = nc.dram_tensor("cos_t", [128, NT * 64], F32, kind="ExternalInput")
    sin_d = nc.dram_tensor("sin_t", [128, NT * 64], F32, kind="ExternalInput")
    q_out = nc.dram_tensor("q_out", [16, TOK, 256], F32, kind="ExternalOutput")
    k_out = nc.dram_tensor("k_out", [16, TOK, 256], F32, kind="ExternalOutput")
    v_out = nc.dram_tensor("v_out", [16, TOK, 128], F32, kind="ExternalOutput")

    # two-byte matmul dtype -> whole next phase's weights prefetch into the
    # second half of the weight pool while the current phase computes
    wbufs = 2 * ND if mmdt in (mybir.dt.float16, mybir.dt.bfloat16) else ND + 2

    with ExitStack() as ctx:
        tc = ctx.enter_context(tile.TileContext(nc))
        w_pool = ctx.enter_context(tc.tile_pool(name="w", bufs=wbufs))
        x_pool = ctx.enter_context(tc.tile_pool(name="x", bufs=3))
        # single-bank psum tiles: finer release granularity, and PE writes to
        # bank b+1 never serialize against ACT/DVE reads of bank b
        ps_pool = ctx.enter_context(tc.tile_pool(name="ps", bufs=8, space="PSUM"))
        a_pool = ctx.enter_context(tc.tile_pool(name="a", bufs=2))
        y_pool = ctx.enter_context(tc.tile_pool(name="y", bufs=2))
        t_pool = ctx.enter_context(tc.tile_pool(name="t", bufs=1))
        s_pool = ctx.enter_context(tc.tile_pool(name="s", bufs=4))
        tab_pool = ctx.enter_context(tc.tile_pool(name="tab", bufs=1))

        cos_t = tab_pool.tile([128, NT * 64], F32, tag="cos")
        sin_t = tab_pool.tile([128, NT * 64], F32, tag="sin")

        # PE warm-up: ~5us of zero matmuls during the DMA lead-in flips the
        # HAM clock gate to 8/8 before the first real matmul arrives
        wu_x = tab_pool.tile([128, 128], mmdt, tag="wux")
        wu_w = tab_pool.tile([128, 512], mmdt, tag="wuw")
        nc.gpsimd.memset(wu_x[:], 0.0)
        nc.gpsimd.memset(wu_w[:], 0.0)
        ps_wu = ps_pool.tile([128, 512], F32, tag="ps")
        for _ in range(6):
            nc.tensor.matmul(ps_wu[:], wu_x[:], wu_w[:], start=True, stop=True)

        outs = {
            "q": q_out.ap().rearrange("h t j -> t h j"),
            "k": k_out.ap().rearrange("h t j -> t h j"),
            "v": v_out.ap().rearrange("h t j -> t h j"),
        }

        for pi, tname in enumerate("qkv"):
            outr = outs[tname]
            xr = dram[f"x{tname}"].ap()
            wts = []

            def load_xt(tt):
                xt = x_pool.tile([128, ND, 128], mmdt, tag="x", name=f"xt{tt}")
                nc.sync.dma_start(
                    xt[:], xr[tt].rearrange("p (dt t) -> p dt t", t=128))
                return xt

            def alloc_ph(i):
                return [
                    ps_pool.tile([128, 512], F32, tag="ps", name=f"ph{i}_{fb}")
                    for fb in range(NFB)
                ]

            def emit_mm_d(ph, xt, d):
                for fb in range(NFB):
                    nc.tensor.matmul(
                        ph[fb][:],
                        xt[:, d, :],
                        wts[d][:, bass.ts(fb, 512)],
                        start=(d == 0),
                        stop=(d == ND - 1),
                    )

            def emit_post(tt, srcs):
                """LN + RoPE + output DMA (q/k) or plain copy-out (v).
                srcs: 4 APs (one per 512-wide f block), PSUM banks or SBUF."""
                if tname == "v":
                    vo = a_pool.tile([128, D], F32, tag="a", name=f"vo{tt}")
                    for fb in range(NFB):
                        nc.scalar.copy(vo[:, bass.ts(fb, 512)], srcs[fb])
                    nc.sync.dma_start(
                        outr[bass.ts(tt, 128)],
                        vo[:].rearrange("p (h j) -> p h j", j=128),
                    )
                    return
                # stats: s = sum_f (ps - c)^2, one partial per psum bank
                # (square results land in `a`, which is overwritten by the
                # normalize pass right after)
                a = a_pool.tile([128, D], F32, tag="a", name=f"a{tt}")
                sc = s_pool.tile([128, NFB], F32, tag="sc", name=f"sc{tt}")
                for fb in range(NFB):
                    nc.scalar.activation(
                        a[:, bass.ts(fb, 512)], srcs[fb], AF.Square,
                        bias=-c_half, scale=1.0, accum_out=sc[:, fb:fb + 1],
                    )
                s = s_pool.tile([128, 1], F32, tag="s", name=f"s{tt}")
                nc.vector.tensor_reduce(
                    s[:], sc[:], axis=mybir.AxisListType.X, op=OP.add)
                u = s_pool.tile([128, 1], F32, tag="u", name=f"u{tt}")
                nc.scalar.activation(u[:], s[:], AF.Sqrt, bias=EPS, scale=1.0 / D)
                r = s_pool.tile([128, 1], F32, tag="r", name=f"r{tt}")
                nc.vector.reciprocal(r[:], u[:])
                # a = (ps - c) * rstd
                for fb in range(NFB):
                    nc.vector.tensor_scalar(
                        a[:, bass.ts(fb, 512)], srcs[fb], c_half, r[:],
                        op0=OP.subtract, op1=OP.mult,
                    )
                # RoPE in place on the first ROT dims of each 256-wide head
                a3 = a[:].rearrange("p (h x) -> p h x", x=256)
                ae = a3[:, :, 0:ROT:2]
                ao = a3[:, :, 1:ROT:2]
                csl = (
                    cos_t[:, bass.ts(tt, 64)]
                    .rearrange("p (u i) -> p u i", u=1)
                    .broadcast_to([128, 8, 64])
                )
                ssl = (
                    sin_t[:, bass.ts(tt, 64)]
                    .rearrange("p (u i) -> p u i", u=1)
                    .broadcast_to([128, 8, 64])
                )
                t1 = t_pool.tile([128, 8, 64], F32, tag="t1", name=f"t1_{tt}")
                t2 = t_pool.tile([128, 8, 64], F32, tag="t2", name=f"t2_{tt}")
                t3 = t_pool.tile([128, 8, 64], F32, tag="t3", name=f"t3_{tt}")
                t4 = t_pool.tile([128, 8, 64], F32, tag="t4", name=f"t4_{tt}")
                nc.vector.tensor_mul(t1[:], ao, ssl)
                nc.vector.tensor_mul(t2[:], ao, csl)
                nc.vector.tensor_mul(t3[:], ae, ssl)
                nc.vector.tensor_mul(t4[:], ae, csl)
                nc.vector.tensor_sub(ae, t4[:], t1[:])
                nc.vector.tensor_add(ao, t2[:], t3[:])
                # second half of the LN output is exactly -a (post-RoPE)
                y2 = y_pool.tile([128, D], F32, tag="y", name=f"y{tt}")
                nc.scalar.mul(y2[:], a[:], -1.0)
                nc.sync.dma_start(
                    outr[bass.ts(tt, 128), 0:8, :],
                    a[:].rearrange("p (h j) -> p h j", j=256),
                )
                nc.sync.dma_start(
                    outr[bass.ts(tt, 128), 8:16, :],
                    y2[:].rearrange("p (h j) -> p h j", j=256),
                )

            # DMA order: lead with the small chunks the very first matmuls
            # need (xt0 d0-1, w0 fb0-1) so the PE starts ~4us earlier, then
            # stream the rest
            paired = pi == 0

            def load_w(d, split=False):
                wt = w_pool.tile([128, D], mmdt, tag="w", name=f"w{tname}{d}")
                src = dram[f"w{tname}"].ap()[bass.ts(d, 128), :]
                if split:
                    nc.sync.dma_start(wt[:, 0:1024], src[:, 0:1024])
                    nc.sync.dma_start(wt[:, 1024:D], src[:, 1024:D])
                else:
                    nc.sync.dma_start(wt[:], src)
                wts.append(wt)

            if paired:
                xt0 = x_pool.tile([128, ND, 128], mmdt, tag="x", name="xt0")
                xr0 = xr[0].rearrange("p (dt t) -> p dt t", t=128)
                nc.sync.dma_start(xt0[:, 0:2, :], xr0[:, 0:2, :])
                load_w(0, split=True)
                nc.sync.dma_start(xt0[:, 2:ND, :], xr0[:, 2:ND, :])
                xts = {0: xt0, 1: load_xt(1)}
                for d in range(1, ND):
                    load_w(d)
            else:
                xts = {0: load_xt(0)}
                for d in range(ND):
                    load_w(d)
            if pi == 0:
                # rope tables: needed only ~20us in, after the first psum tile
                nc.sync.dma_start(cos_t[:], cos_d.ap())
                nc.sync.dma_start(sin_t[:], sin_d.ap())

            if paired:
                # The first phase is paced by weight DMA arrival (one d-tile
                # per ~1.4us vs 0.86us of PE work per tile). Interleave the
                # first TWO token tiles per d step (their 8 psum banks fill
                # PSUM exactly), so the PE does 1.7us of work per arriving
                # weight tile and never idles on the load.
                ph0, ph1 = alloc_ph(0), alloc_ph(1)
                for d in range(ND):
                    emit_mm_d(ph0, xts[0], d)
                    emit_mm_d(ph1, xts[1], d)
                # t2 reuses these banks: release them via fast alternating
                # ACT/DVE copies instead of waiting for the stats->normalize
                # chain, then run LN off the SBUF copies
                z0 = a_pool.tile([128, D], F32, tag="z", bufs=2, name="z0")
                z1 = a_pool.tile([128, D], F32, tag="z", bufs=2, name="z1")
                for z, ph in ((z0, ph0), (z1, ph1)):
                    for fb in range(NFB):
                        dst = z[:, bass.ts(fb, 512)]
                        if fb % 2 == 0:
                            nc.scalar.copy(dst, ph[fb][:])
                        else:
                            nc.vector.tensor_copy(dst, ph[fb][:])
                emit_post(0, [z0[:, bass.ts(fb, 512)] for fb in range(NFB)])
                emit_post(1, [z1[:, bass.ts(fb, 512)] for fb in range(NFB)])

            for tt in range(2 if paired else 0, NT):
                xt = xts.get(tt) or load_xt(tt)
                ph = alloc_ph(tt)
                if tname == "v" and tt == NT - 1:
                    # fb-outer on the final tile: each psum bank finishes
                    # ~3.5us apart, so its copy+DMA overlaps the remaining
                    # banks' matmuls instead of serializing after the last one
                    vo = a_pool.tile([128, D], F32, tag="a")
                    for fb in range(NFB):
                        for d in range(ND):
                            nc.tensor.matmul(
                                ph[fb][:],
                                xt[:, d, :],
                                wts[d][:, bass.ts(fb, 512)],
                                start=(d == 0),
                                stop=(d == ND - 1),
                            )
                        nc.scalar.copy(vo[:, bass.ts(fb, 512)], ph[fb][:])
                        nc.sync.dma_start(
                            outr[bass.ts(tt, 128), 4 * fb:4 * (fb + 1), :],
                            vo[:, bass.ts(fb, 512)].rearrange(
                                "p (h j) -> p h j", j=128),
                        )
                    continue
                for d in range(ND):
                    emit_mm_d(ph, xt, d)
                emit_post(tt, [ph[fb][:] for fb in range(NFB)])
    nc.compile()
    return nc


_build_cache: dict = {}


def _mm_dtype():
    import os
    name = os.environ.get("BAKA_MM_DTYPE", "float16")
    return {
        "float16": mybir.dt.float16,
        "bfloat16": mybir.dt.bfloat16,
        "float32r": F32R,
    }[name]


def _to_mm(a: np.ndarray, mmdt) -> np.ndarray:
    if mmdt == F32R:
        return _round_fp32r(a)
    if mmdt == mybir.dt.float16:
        return np.ascontiguousarray(a, np.float32).astype(np.float16)
    import ml_dtypes
    return np.ascontiguousarray(a, np.float32).astype(ml_dtypes.bfloat16)


def _get_nc(c_half: float):
    key = (c_half, _mm_dtype())
    nc = _build_cache.get(key)
    if nc is None:
        nc = _build(c_half, _mm_dtype())
        _build_cache.clear()
        _build_cache[key] = nc
    return nc


def _ensure_ntff_hook():
    """Install antenv.axon_hooks NTFF profile hook if the image lacks it."""
    import types
    import antenv

    if getattr(antenv, "axon_hooks", None) is not None:
        return
    mod = types.ModuleType("antenv.axon_hooks")
    state = {"h": None}
    mod.set_axon_ntff_profile_hook = lambda h: state.__setitem__("h", h)
    mod.get_axon_ntff_profile_hook = lambda: state["h"]
    sys.modules["antenv.axon_hooks"] = mod
    antenv.axon_hooks = mod
    try:
        from trn_agent_boot.trn_boot import _ntff_profile_via_ctypes
        so = "/opt/axon/libaxon_pjrt.so"
        import os
        if os.path.exists(so):
            mod.set_axon_ntff_profile_hook(_ntff_profile_via_ctypes(so))
    except Exception:
        pass


def _numpy_fallback(q_state, k_state, v_state, Wq, Wk, Wv, negator,
                    q_gamma, q_beta, k_gamma, k_beta, offset):
    """Reference math in numpy — used only for non-canonical gamma/beta."""
    def ln(x, g, b):
        mu = x.mean(-1, keepdims=True)
        var = ((x - mu) ** 2).mean(-1, keepdims=True)
        return (x - mu) / np.sqrt(var + EPS) * g + b

    def heads(x, n):
        b, t, d = x.shape
        return x.reshape(b, t, n, d // n).transpose(0, 2, 1, 3)

    def rope(x, off):
        t = (np.arange(x.shape[-2], dtype=np.float32) + np.float32(off))
        inv = (1.0 / (10000.0 ** (np.arange(0, ROT, 2, dtype=np.float32) / np.float32(ROT)))).astype(np.float32)
        fr = np.repeat(t[:, None] * inv[None, :], 2, axis=-1)
        cos, sin = np.cos(fr), np.sin(fr)
        xl, xr = x[..., :ROT], x[..., ROT:]
        x2 = xl.reshape(*xl.shape[:-1], ROT // 2, 2)
        rot = np.stack((-x2[..., 1], x2[..., 0]), axis=-1).reshape(xl.shape)
        return np.concatenate([xl * cos + rot * sin, xr], axis=-1)

    q = np.einsum("btd,fd->btf", q_state, Wq)
    k = np.einsum("btd,fd->btf", k_state, Wk)
    v = np.einsum("btd,fd->btf", v_state, Wv)
    q = np.concatenate((q, negator - q), axis=-1)
    k = np.concatenate((k, negator - k), axis=-1)
    q = ln(q, q_gamma, q_beta)
    k = ln(k, k_gamma, k_beta)
    q, k, v = heads(q, 16), heads(k, 16), heads(v, 16)
    return (rope(q, offset).astype(np.float32), rope(k, offset).astype(np.float32),
            v.astype(np.float32))


def kernel(q_state, k_state, v_state, Wq, Wk, Wv, negator,
           q_gamma, q_beta, k_gamma, k_beta, offset, **_unused):
    global last_exec_time_ns, last_results
    import os

    q_state = np.asarray(q_state, np.float32)
    k_state = np.asarray(k_state, np.float32)
    v_state = np.asarray(v_state, np.float32)
    q_gamma = np.asarray(q_gamma, np.float32)
    q_beta = np.asarray(q_beta, np.float32)
    k_gamma = np.asarray(k_gamma, np.float32)
    k_beta = np.asarray(k_beta, np.float32)
    negator_f = float(np.asarray(negator, np.float32))
    offset_i = int(np.asarray(offset))

    trivial_affine = (
        np.all(q_gamma == 1.0) and np.all(k_gamma == 1.0)
        and np.all(q_beta == 0.0) and np.all(k_beta == 0.0)
    )
    if not trivial_affine:
        return _numpy_fallback(q_state, k_state, v_state,
                               np.asarray(Wq, np.float32), np.asarray(Wk, np.float32),
                               np.asarray(Wv, np.float32), negator_f,
                               q_gamma, q_beta, k_gamma, k_beta, offset_i)

    c_half = negator_f / 2.0

    # host-side prep (layout + matmul-dtype rounding + rope tables)
    mmdt = _mm_dtype()
    wqT = _to_mm(np.asarray(Wq, np.float32).T, mmdt)
    wkT = _to_mm(np.asarray(Wk, np.float32).T, mmdt)
    wvT = _to_mm(np.asarray(Wv, np.float32).T, mmdt)
    flats = {
        "q": q_state.reshape(B * T, D),
        "k": k_state.reshape(B * T, D),
        "v": v_state.reshape(B * T, D),
    }

    inv = (1.0 / (10000.0 ** (np.arange(0, ROT, 2, dtype=np.float32)
                              / np.float32(ROT)))).astype(np.float32)

    in_maps = []
    for c in range(NCORES):
        m = {"wq": wqT, "wk": wkT, "wv": wvT}
        rows = slice(c * TOK, (c + 1) * TOK)
        for t in "qkv":
            # pack [tok, d] chunk -> [tt, p, dt*128+tl] with p the
            # contraction index within tile dt and tl the token within tile tt
            xm = _to_mm(flats[t][rows], mmdt)
            m[f"x{t}"] = np.ascontiguousarray(
                xm.reshape(NT, 128, ND, 128).transpose(0, 3, 2, 1)
            ).reshape(NT, 128, D)
        t0 = (c % (T // TOK)) * TOK
        tpos = (np.arange(t0, t0 + TOK, dtype=np.float32)
                + np.float32(offset_i)).astype(np.float32)
        ang = tpos[:, None] * inv[None, :]                       # [TOK, 64] f32
        cos_p = np.cos(ang).astype(np.float32)
        sin_p = np.sin(ang).astype(np.float32)
        m["cos_t"] = np.ascontiguousarray(
            cos_p.reshape(NT, 128, 64).transpose(1, 0, 2).reshape(128, NT * 64))
        m["sin_t"] = np.ascontiguousarray(
            sin_p.reshape(NT, 128, 64).transpose(1, 0, 2).reshape(128, NT * 64))
        in_maps.append(m)

    nc = _get_nc(c_half)
    trace = bool(int(os.environ.get("BAKA_TRACE", "0")))
    kw = {}
    if trace:
        _ensure_ntff_hook()
        kw = {"trace": True, "trace_cores": list(range(NCORES))}
    res = run_bass_kernel_spmd(nc, in_maps, list(range(NCORES)), **kw)
    last_exec_time_ns = res.exec_time_ns
    last_results = res

    q = np.empty((B, 16, T, 256), np.float32)
    k = np.empty((B, 16, T, 256), np.float32)
    v = np.empty((B, 16, T, 128), np.float32)
    for c in range(NCORES):
        b = c // (T // TOK)
        t0 = (c % (T // TOK)) * TOK
        q[b, :, t0:t0 + TOK] = res.results[c]["q_out"]
        k[b, :, t0:t0 + TOK] = res.results[c]["k_out"]
        v[b, :, t0:t0 + TOK] = res.results[c]["v_out"]
    return q, k, v


# revision 30
# speedup vs baseline: 1.0062x; 1.0062x over previous
"""Trainium2 Bass kernel for BakaAttentionQKV.

Computes, for full inputs (B=4, T=2048, D=2048, 16 heads):
    q = q_state @ Wq.T ; k = k_state @ Wk.T ; v = v_state @ Wv.T
    q,k -> concat(x, negator - x) -> LayerNorm(gamma, beta) -> split heads -> RoPE(first 128 dims)
    v -> split heads
Returns (q [4,16,2048,256], k [4,16,2048,256], v [4,16,2048,128]).

Key algebraic simplification: for z = concat(x, negator - x),
mean(z) = negator/2 exactly, and var(z) = mean_f((x - negator/2)^2), so the
LayerNorm only needs the first half: n = (x - c) * rsqrt(var + eps) with
c = negator/2; the second half of the normalized vector is exactly -n.
With gamma == 1 and beta == 0 (the canonical inputs) the full LN output is
[n, -n], so the matmul for the second half is free.

Sharding: 8-way data parallel over the 8192 (b, t) token rows; each core gets
1024 contiguous rows (= half of one batch element) and computes all heads.
Matmuls run in fp16 (10-bit mantissa; full 1 cycle/row PE rate and half the
HBM traffic of fp32) with fp32 PSUM accumulation; x/W are converted on the
host. BAKA_MM_DTYPE=float32r selects a TF32-like fp32r path instead (slightly
more accurate, ~45% slower end-to-end because weights can't double-buffer).
Measured vs the fp64-ish jax reference: rel2 ~2.5e-4.
"""
import sys

sys.path.insert(0, "/opt/trn_rl_repo")

from contextlib import ExitStack

import numpy as np

import concourse.bacc as bacc
import concourse.bass as bass
import concourse.tile as tile
from concourse import mybir
from concourse.bass_utils import run_bass_kernel_spmd

F32 = mybir.dt.float32
F32R = mybir.dt.float32r
AF = mybir.ActivationFunctionType
OP = mybir.AluOpType

B, T, D = 4, 2048, 2048
NCORES = 8
TOK = B * T // NCORES          # 1024 token rows per core
NT = TOK // 128                # 8 token tiles per core
ND = D // 128                  # 16 contraction tiles
NFB = D // 512                 # 4 psum banks per row block
EPS = 1e-5
ROT = 128                      # rotary dims per head

# diagnostics for test harness
last_exec_time_ns = None
last_results = None


def _round_fp32r(a: np.ndarray) -> np.ndarray:
    """Round fp32 -> fp32r (11-bit mantissa, low 12 bits zero), RNE."""
    u = np.ascontiguousarray(a, dtype=np.float32).view(np.uint32).copy()
    u += np.uint32(0x7FF) + ((u >> np.uint32(12)) & np.uint32(1))
    u &= np.uint32(0xFFFFF000)
    return u.view(np.float32)


def _build(c_half: float, mmdt=None):
    if mmdt is None:
        mmdt = mybir.dt.float16
    nc = bacc.Bacc("TRN2", target_bir_lowering=False, debug=False)
    # register const APs needed by scalar.activation float biases
    for val in (-c_half, EPS):
        if (F32, val) not in nc.const_aps.aps:
            t = nc.alloc_sbuf_tensor(f"const-f32-{val}", [128, 1], F32)
            nc.gpsimd.memset(t.ap(), val)
            nc.const_aps.aps[(F32, val)] = t.ap()
    nc.all_engine_barrier()
    dram = {}
    for t in "qkv":
        # x pre-packed host-side as [tt, p, dt*128+tl]: each token tile's
        # stationary block loads as one contiguous 2D DMA
        dram[f"x{t}"] = nc.dram_tensor(f"x{t}", [NT, 128, D], mmdt, kind="ExternalInput")
        dram[f"w{t}"] = nc.dram_tensor(f"w{t}", [D, D], mmdt, kind="ExternalInput")
    cos_d = nc.dram_tensor("cos_t", [128, NT * 64], F32, kind="ExternalInput")
    sin_d = nc.dram_tensor("sin_t", [128, NT * 64], F32, kind="ExternalInput")
    q_out = nc.dram_tensor("q_out", [16, TOK, 256], F32, kind="ExternalOutput")
    k_out = nc.dram_tensor("k_out", [16, TOK, 256], F32, kind="ExternalOutput")
    v_out = nc.dram_tensor("v_out", [16, TOK, 128], F32, kind="ExternalOutput")

    # two-byte matmul dtype -> whole next phase's weights prefetch into the
    # second half of the weight pool while the current phase computes
    wbufs = 2 * ND if mmdt in (mybir.dt.float16, mybir.dt.bfloat16) else ND + 2

    with ExitStack() as ctx:
        tc = ctx.enter_context(tile.TileContext(nc))
        w_pool = ctx.enter_context(tc.tile_pool(name="w", bufs=wbufs))
        x_pool = ctx.enter_context(tc.tile_pool(name="x", bufs=3))
        # single-bank psum tiles: finer release granularity, and PE writes to
        # bank b+1 never serialize against ACT/DVE reads of bank b
        ps_pool = ctx.enter_context(tc.tile_pool(name="ps", bufs=8, space="PSUM"))
        a_pool = ctx.enter_context(tc.tile_pool(name="a", bufs=2))
        y_pool = ctx.enter_context(tc.tile_pool(name="y", bufs=2))
        t_pool = ctx.enter_context(tc.tile_pool(name="t", bufs=1))
        s_pool = ctx.enter_context(tc.tile_pool(name="s", bufs=4))
        tab_pool = ctx.enter_context(tc.tile_pool(name="tab", bufs=1))

        cos_t = tab_pool.tile([128, NT * 64], F32, tag="cos")
        sin_t = tab_pool.tile([128, NT * 64], F32, tag="sin")

        # PE warm-up: ~5us of zero matmuls during the DMA lead-in flips the
        # HAM clock gate to 8/8 before the first real matmul arrives
        wu_x = tab_pool.tile([128, 128], mmdt, tag="wux")
        wu_w = tab_pool.tile([128, 512], mmdt, tag="wuw")
        nc.gpsimd.memset(wu_x[:], 0.0)
        nc.gpsimd.memset(wu_w[:], 0.0)
        ps_wu = ps_pool.tile([128, 512], F32, tag="ps")
        for _ in range(18):
            nc.tensor.matmul(ps_wu[:], wu_x[:], wu_w[:], start=True, stop=True)

        outs = {
            "q": q_out.ap().rearrange("h t j -> t h j"),
            "k": k_out.ap().rearrange("h t j -> t h j"),
            "v": v_out.ap().rearrange("h t j -> t h j"),
        }

        for pi, tname in enumerate("qkv"):
            outr = outs[tname]
            xr = dram[f"x{tname}"].ap()
            wts = []

            def load_xt(tt):
                xt = x_pool.tile([128, ND, 128], mmdt, tag="x", name=f"xt{tt}")
                nc.sync.dma_start(
                    xt[:], xr[tt].rearrange("p (dt t) -> p dt t", t=128))
                return xt

            def alloc_ph(i):
                return [
                    ps_pool.tile([128, 512], F32, tag="ps", name=f"ph{i}_{fb}")
                    for fb in range(NFB)
                ]

            def emit_mm_d(ph, xt, d):
                for fb in range(NFB):
                    nc.tensor.matmul(
                        ph[fb][:],
                        xt[:, d, :],
                        wts[d][:, bass.ts(fb, 512)],
                        start=(d == 0),
                        stop=(d == ND - 1),
                    )

            def emit_post(tt, srcs):
                """LN + RoPE + output DMA (q/k) or plain copy-out (v).
                srcs: 4 APs (one per 512-wide f block), PSUM banks or SBUF."""
                if tname == "v":
                    vo = a_pool.tile([128, D], F32, tag="a", name=f"vo{tt}")
                    for fb in range(NFB):
                        nc.scalar.copy(vo[:, bass.ts(fb, 512)], srcs[fb])
                    nc.sync.dma_start(
                        outr[bass.ts(tt, 128)],
                        vo[:].rearrange("p (h j) -> p h j", j=128),
                    )
                    return
                # stats: s = sum_f (ps - c)^2, one partial per psum bank
                # (square results land in `a`, which is overwritten by the
                # normalize pass right after)
                a = a_pool.tile([128, D], F32, tag="a", name=f"a{tt}")
                sc = s_pool.tile([128, NFB], F32, tag="sc", name=f"sc{tt}")
                for fb in range(NFB):
                    nc.scalar.activation(
                        a[:, bass.ts(fb, 512)], srcs[fb], AF.Square,
                        bias=-c_half, scale=1.0, accum_out=sc[:, fb:fb + 1],
                    )
                s = s_pool.tile([128, 1], F32, tag="s", name=f"s{tt}")
                nc.vector.tensor_reduce(
                    s[:], sc[:], axis=mybir.AxisListType.X, op=OP.add)
                u = s_pool.tile([128, 1], F32, tag="u", name=f"u{tt}")
                nc.scalar.activation(u[:], s[:], AF.Sqrt, bias=EPS, scale=1.0 / D)
                r = s_pool.tile([128, 1], F32, tag="r", name=f"r{tt}")
                nc.vector.reciprocal(r[:], u[:])
                # a = (ps - c) * rstd
                for fb in range(NFB):
                    nc.vector.tensor_scalar(
                        a[:, bass.ts(fb, 512)], srcs[fb], c_half, r[:],
                        op0=OP.subtract, op1=OP.mult,
                    )
                # RoPE in place on the first ROT dims of each 256-wide head
                a3 = a[:].rearrange("p (h x) -> p h x", x=256)
                ae = a3[:, :, 0:ROT:2]
                ao = a3[:, :, 1:ROT:2]
                csl = (
                    cos_t[:, bass.ts(tt, 64)]
                    .rearrange("p (u i) -> p u i", u=1)
                    .broadcast_to([128, 8, 64])
                )
                ssl = (
                    sin_t[:, bass.ts(tt, 64)]
                    .rearrange("p (u i) -> p u i", u=1)
                    .broadcast_to([128, 8, 64])
                )
                t1 = t_pool.tile([128, 8, 64], F32, tag="t1", name=f"t1_{tt}")
                t2 = t_pool.tile([128, 8, 64], F32, tag="t2", name=f"t2_{tt}")
                t3 = t_pool.tile([128, 8, 64], F32, tag="t3", name=f"t3_{tt}")
                t4 = t_pool.tile([128, 8, 64], F32, tag="t4", name=f"t4_{tt}")
                nc.vector.tensor_mul(t1[:], ao, ssl)
                nc.vector.tensor_mul(t2[:], ao, csl)
                nc.vector.tensor_mul(t3[:], ae, ssl)
                nc.vector.tensor_mul(t4[:], ae, csl)
                nc.vector.tensor_sub(ae, t4[:], t1[:])
                nc.vector.tensor_add(ao, t2[:], t3[:])
                # second half of the LN output is exactly -a (post-RoPE)
                y2 = y_pool.tile([128, D], F32, tag="y", name=f"y{tt}")
                nc.scalar.mul(y2[:], a[:], -1.0)
                nc.sync.dma_start(
                    outr[bass.ts(tt, 128), 0:8, :],
                    a[:].rearrange("p (h j) -> p h j", j=256),
                )
                nc.sync.dma_start(
                    outr[bass.ts(tt, 128), 8:16, :],
                    y2[:].rearrange("p (h j) -> p h j", j=256),
                )

            # DMA order: xt0, w0, xt1, w1.. — the very first matmul needs only
            # xt0 + w0, so the PE starts as early as possible
            paired = pi == 0
            xts = {0: load_xt(0)}

            def load_w(d):
                wt = w_pool.tile([128, D], mmdt, tag="w", name=f"w{tname}{d}")
                nc.sync.dma_start(wt[:], dram[f"w{tname}"].ap()[bass.ts(d, 128), :])
                wts.append(wt)

            load_w(0)
            if paired:
                xts[1] = load_xt(1)
            for d in range(1, ND):
                load_w(d)
            if pi == 0:
                # rope tables: needed only ~20us in, after the first psum tile
                nc.sync.dma_start(cos_t[:], cos_d.ap())
                nc.sync.dma_start(sin_t[:], sin_d.ap())

            if paired:
                # The first phase is paced by weight DMA arrival (one d-tile
                # per ~1.4us vs 0.86us of PE work per tile). Interleave the
                # first TWO token tiles per d step (their 8 psum banks fill
                # PSUM exactly), so the PE does 1.7us of work per arriving
                # weight tile and never idles on the load.
                ph0, ph1 = alloc_ph(0), alloc_ph(1)
                for d in range(ND):
                    emit_mm_d(ph0, xts[0], d)
                    emit_mm_d(ph1, xts[1], d)
                # t2 reuses these banks: release them via fast alternating
                # ACT/DVE copies instead of waiting for the stats->normalize
                # chain, then run LN off the SBUF copies
                z0 = a_pool.tile([128, D], F32, tag="z", bufs=2, name="z0")
                z1 = a_pool.tile([128, D], F32, tag="z", bufs=2, name="z1")
                for z, ph in ((z0, ph0), (z1, ph1)):
                    for fb in range(NFB):
                        dst = z[:, bass.ts(fb, 512)]
                        if fb % 2 == 0:
                            nc.scalar.copy(dst, ph[fb][:])
                        else:
                            nc.vector.tensor_copy(dst, ph[fb][:])
                emit_post(0, [z0[:, bass.ts(fb, 512)] for fb in range(NFB)])
                emit_post(1, [z1[:, bass.ts(fb, 512)] for fb in range(NFB)])

            for tt in range(2 if paired else 0, NT):
                xt = xts.get(tt) or load_xt(tt)
                ph = alloc_ph(tt)
                if tname == "v" and tt == NT - 1:
                    # fb-outer on the final tile: each psum bank finishes
                    # ~3.5us apart, so its copy+DMA overlaps the remaining
                    # banks' matmuls instead of serializing after the last one
                    vo = a_pool.tile([128, D], F32, tag="a")
                    for fb in range(NFB):
                        for d in range(ND):
                            nc.tensor.matmul(
                                ph[fb][:],
                                xt[:, d, :],
                                wts[d][:, bass.ts(fb, 512)],
                                start=(d == 0),
                                stop=(d == ND - 1),
                            )
                        nc.scalar.copy(vo[:, bass.ts(fb, 512)], ph[fb][:])
                        nc.sync.dma_start(
                            outr[bass.ts(tt, 128), 4 * fb:4 * (fb + 1), :],
                            vo[:, bass.ts(fb, 512)].rearrange(
                                "p (h j) -> p h j", j=128),
                        )
                    continue
                for d in range(ND):
                    emit_mm_d(ph, xt, d)
                emit_post(tt, [ph[fb][:] for fb in range(NFB)])
    nc.compile()
    return nc


_build_cache: dict = {}


def _mm_dtype():
    import os
    name = os.environ.get("BAKA_MM_DTYPE", "float16")
    return {
        "float16": mybir.dt.float16,
        "bfloat16": mybir.dt.bfloat16,
        "float32r": F32R,
    }[name]


def _to_mm(a: np.ndarray, mmdt) -> np.ndarray:
    if mmdt == F32R:
        return _round_fp32r(a)
    if mmdt == mybir.dt.float16:
        return np.ascontiguousarray(a, np.float32).astype(np.float16)
    import ml_dtypes
    return np.ascontiguousarray(a, np.float32).astype(ml_dtypes.bfloat16)


def _get_nc(c_half: float):
    key = (c_half, _mm_dtype())
    nc = _build_cache.get(key)
    if nc is None:
        nc = _build(c_half, _mm_dtype())
        _build_cache.clear()
        _build_cache[key] = nc
    return nc


def _ensure_ntff_hook():
    """Install antenv.axon_hooks NTFF profile hook if the image lacks it."""
    import types
    import antenv

    if getattr(antenv, "axon_hooks", None) is not None:
        return
    mod = types.ModuleType("antenv.axon_hooks")
    state = {"h": None}
    mod.set_axon_ntff_profile_hook = lambda h: state.__setitem__("h", h)
    mod.get_axon_ntff_profile_hook = lambda: state["h"]
    sys.modules["antenv.axon_hooks"] = mod
    antenv.axon_hooks = mod
    try:
        from trn_agent_boot.trn_boot import _ntff_profile_via_ctypes
        so = "/opt/axon/libaxon_pjrt.so"
        import os
        if os.path.exists(so):
            mod.set_axon_ntff_profile_hook(_ntff_profile_via_ctypes(so))
    except Exception:
        pass


def _numpy_fallback(q_state, k_state, v_state, Wq, Wk, Wv, negator,
                    q_gamma, q_beta, k_gamma, k_beta, offset):
    """Reference math in numpy — used only for non-canonical gamma/beta."""
    def ln(x, g, b):
        mu = x.mean(-1, keepdims=True)
        var = ((x - mu) ** 2).mean(-1, keepdims=True)
        return (x - mu) / np.sqrt(var + EPS) * g + b

    def heads(x, n):
        b, t, d = x.shape
        return x.reshape(b, t, n, d // n).transpose(0, 2, 1, 3)

    def rope(x, off):
        t = (np.arange(x.shape[-2], dtype=np.float32) + np.float32(off))
        inv = (1.0 / (10000.0 ** (np.arange(0, ROT, 2, dtype=np.float32) / np.float32(ROT)))).astype(np.float32)
        fr = np.repeat(t[:, None] * inv[None, :], 2, axis=-1)
        cos, sin = np.cos(fr), np.sin(fr)
        xl, xr = x[..., :ROT], x[..., ROT:]
        x2 = xl.reshape(*xl.shape[:-1], ROT // 2, 2)
        rot = np.stack((-x2[..., 1], x2[..., 0]), axis=-1).reshape(xl.shape)
        return np.concatenate([xl * cos + rot * sin, xr], axis=-1)

    q = np.einsum("btd,fd->btf", q_state, Wq)
    k = np.einsum("btd,fd->btf", k_state, Wk)
    v = np.einsum("btd,fd->btf", v_state, Wv)
    q = np.concatenate((q, negator - q), axis=-1)
    k = np.concatenate((k, negator - k), axis=-1)
    q = ln(q, q_gamma, q_beta)
    k = ln(k, k_gamma, k_beta)
    q, k, v = heads(q, 16), heads(k, 16), heads(v, 16)
    return (rope(q, offset).astype(np.float32), rope(k, offset).astype(np.float32),
            v.astype(np.float32))


def kernel(q_state, k_state, v_state, Wq, Wk, Wv, negator,
           q_gamma, q_beta, k_gamma, k_beta, offset, **_unused):
    global last_exec_time_ns, last_results
    import os

    q_state = np.asarray(q_state, np.float32)
    k_state = np.asarray(k_state, np.float32)
    v_state = np.asarray(v_state, np.float32)
    q_gamma = np.asarray(q_gamma, np.float32)
    q_beta = np.asarray(q_beta, np.float32)
    k_gamma = np.asarray(k_gamma, np.float32)
    k_beta = np.asarray(k_beta, np.float32)
    negator_f = float(np.asarray(negator, np.float32))
    offset_i = int(np.asarray(offset))

    trivial_affine = (
        np.all(q_gamma == 1.0) and np.all(k_gamma == 1.0)
        and np.all(q_beta == 0.0) and np.all(k_beta == 0.0)
    )
    if not trivial_affine:
        return _numpy_fallback(q_state, k_state, v_state,
                               np.asarray(Wq, np.float32), np.asarray(Wk, np.float32),
                               np.asarray(Wv, np.float32), negator_f,
                               q_gamma, q_beta, k_gamma, k_beta, offset_i)

    c_half = negator_f / 2.0

    # host-side prep (layout + matmul-dtype rounding + rope tables)
    mmdt = _mm_dtype()
    wqT = _to_mm(np.asarray(Wq, np.float32).T, mmdt)
    wkT = _to_mm(np.asarray(Wk, np.float32).T, mmdt)
    wvT = _to_mm(np.asarray(Wv, np.float32).T, mmdt)
    flats = {
        "q": q_state.reshape(B * T, D),
        "k": k_state.reshape(B * T, D),
        "v": v_state.reshape(B * T, D),
    }

    inv = (1.0 / (10000.0 ** (np.arange(0, ROT, 2, dtype=np.float32)
                              / np.float32(ROT)))).astype(np.float32)

    in_maps = []
    for c in range(NCORES):
        m = {"wq": wqT, "wk": wkT, "wv": wvT}
        rows = slice(c * TOK, (c + 1) * TOK)
        for t in "qkv":
            # pack [tok, d] chunk -> [tt, p, dt*128+tl] with p the
            # contraction index within tile dt and tl the token within tile tt
            xm = _to_mm(flats[t][rows], mmdt)
            m[f"x{t}"] = np.ascontiguousarray(
                xm.reshape(NT, 128, ND, 128).transpose(0, 3, 2, 1)
            ).reshape(NT, 128, D)
        t0 = (c % (T // TOK)) * TOK
        tpos = (np.arange(t0, t0 + TOK, dtype=np.float32)
                + np.float32(offset_i)).astype(np.float32)
        ang = tpos[:, None] * inv[None, :]                       # [TOK, 64] f32
        cos_p = np.cos(ang).astype(np.float32)
        sin_p = np.sin(ang).astype(np.float32)
        m["cos_t"] = np.ascontiguousarray(
            cos_p.reshape(NT, 128, 64).transpose(1, 0, 2).reshape(128, NT * 64))
        m["sin_t"] = np.ascontiguousarray(
            sin_p.reshape(NT, 128, 64).transpose(1, 0, 2).reshape(128, NT * 64))
        in_maps.append(m)

    nc = _get_nc(c_half)
    trace = bool(int(os.environ.get("BAKA_TRACE", "0")))
    kw = {}
    if trace:
        _ensure_ntff_hook()
        kw = {"trace": True, "trace_cores": list(range(NCORES))}
    res = run_bass_kernel_spmd(nc, in_maps, list(range(NCORES)), **kw)
    last_exec_time_ns = res.exec_time_ns
    last_results = res

    q = np.empty((B, 16, T, 256), np.float32)
    k = np.empty((B, 16, T, 256), np.float32)
    v = np.empty((B, 16, T, 128), np.float32)
    for c in range(NCORES):
        b = c // (T // TOK)
        t0 = (c % (T // TOK)) * TOK
        q[b, :, t0:t0 + TOK] = res.results[c]["q_out"]
        k[b, :, t0:t0 + TOK] = res.results[c]["k_out"]
        v[b, :, t0:t0 + TOK] = res.results[c]["v_out"]
    return q, k, v
